# revision 19
# baseline (speedup 1.0000x reference)
"""Trainium2 Bass kernel for nn_ClassChannelAttention.

Computes: out = x * scale[None, :, None, None] where
  scale[c] = sum_k softmax(channel_attention, axis=-1)[k, c]

Sharding: data-parallel over batch B=16 across 8 cores (2 batches/core);
channel_attention (150, 768) replicated to every core. The softmax+class-sum
is tiny and recomputed on each core (no collectives needed).

Precision: the kernel streams x in/out as bf16 (host converts fp32->bf16 on
the way in and upcasts on the way out; the channel scale itself stays fp32
end-to-end on device). This halves HBM traffic per core (50.3 MB -> 25.2 MB)
at a ~2.3e-3 rel-l2 cost, far under the 2e-2 gate.

DMA regime (measured across five variants): per-core HBM throughput is
~220 GB/s while the stream is read-only and ~360-400 GB/s once loads and
stores interleave, roughly independent of descriptor geometry between
24 KiB and 48 KiB rows (32 KiB rows / (96, 16384) tiles measured best
end-to-end). So the schedule (a) keeps the proven 32 KiB-row geometry,
(b) makes the scale preamble as short as possible, and (c) splits the FIRST
tile 64+32 rows so the first store enters the ring ~12us earlier — mixing
starts at the earliest point the scale pipeline allows. Big DMAs are
bitcast to float32 (same bytes, 4-byte-typed descriptors). Loads ride the
Sync HWDGE ring, stores the Scalar ring. The channel_attention load is
issued FIRST on the Sync ring: rings drain FIFO so it lands in ~3us; on the
other ring it would round-robin packet-by-packet against bulk x loads and
not land for ~30us, stalling everything (measured).

Layout: x viewed as (384, 16384) bf16 — FOUR consecutive channels per
32 KiB partition row — in 4 tiles of (96, 16384); tile i covers
channel-quads q = 96*(i%2)+p, so quarter m of tile i is scaled by
scales_all[p, 4*(i%2)+m] where scales_all[p, 4h+m] = scale[4*(96h+p)+m].
Tile 0 is processed as 64+32-row sub-tiles (partition bases 0 and 64 —
engine ops only accept bases 0/32/64/96, and the scale column stays
lane-aligned since sub-ranges of partitions map to the same q sub-ranges).

Scale pipeline (~13us): channel_attention loads as (75, 1536) via two
75-row DMAs (partition p holds classes p and 75+p); exp per class-half on
ACT (no max-subtraction: ca is N(0,1), fp32 exp cannot overflow) with fused
row-sums; one DVE reciprocal [75,2]; then softmax normalization and
class-sum fold into 16 tiny PE matmuls accumulating the two class halves:
bigpsum[:, 512*(4h+m)] = sum_p e2[p, 768*rnd + 4*(96h+q) + m] * r2[p, rnd]
(lhsT = strided e2 view, rhs = reciprocal column). Each (h, m) output sits
in its own PSUM bank: accumulation groups are bank-granular, concurrent
groups must live in distinct banks (column-sliced groups in one bank
corrupt the sums — caught by CoreSim). One strided ACT copy moves the bank
columns to SBUF fp32: the DVE tensor_scalar scalar must be SBUF-sourced to
keep the 4x_2p packed mode (a PSUM-sourced scalar drops the multiply to 1x
on HW — measured). Quarter-multiplies: bf16, step-1, 4B-aligned -> DVE
4x_2p, ~1.1us each, 20 total, hidden under the DMA window.
"""

import numpy as np
import ml_dtypes

import concourse.bacc as bacc
import concourse.mybir as mybir
import concourse.tile as tile
from concourse import bass_utils

N_CORES = 8
B, C, H, W = 16, 768, 64, 64
K_CLS = 150
B_SH = B // N_CORES          # 2 batches per core
F = H * W                    # 4096
CPP = 4                      # channels packed per partition row (32 KiB bf16)
ROWS4 = B_SH * C // CPP      # 384 rows in the merged view
P_T = 96                     # partitions per tile
N_TILES = ROWS4 // P_T       # 4 tiles of (96, 16384) per core
F4 = CPP * F                 # 16384
KH = K_CLS // 2              # 75: two classes per partition
PSUM_BANK = 512              # fp32 elems per PSUM bank per partition
X_BUFS = 4                   # all 4 x tiles in flight

_module_cache = {}


def _body(tc, out, x, ca):
    nc = tc.nc
    f32 = mybir.dt.float32
    Exp = mybir.ActivationFunctionType.Exp

    with (
        tc.tile_pool(name="attn", bufs=1) as attn_pool,
        tc.tile_pool(name="small", bufs=1) as small,
        tc.tile_pool(name="psum", bufs=1, space="PSUM") as psum_pool,
        tc.tile_pool(name="xt", bufs=X_BUFS) as xpool,
    ):
        # scales_all[p, 4h+m] = sum-softmax over channel 4*(96h+p) + m.
        scales_all = small.tile([P_T, 2 * CPP], f32)
        bigpsum = psum_pool.tile([P_T, 2 * CPP * PSUM_BANK], f32)

        fdma = mybir.dt.float32  # bitcast target for big DMAs (same bytes)
        xf = (
            x.rearrange("b c h w -> (b c) (h w)")
            .rearrange("(a four) f -> a (four f)", four=CPP)
            .bitcast(fdma)
        )
        of = (
            out.rearrange("b c h w -> (b c) (h w)")
            .rearrange("(a four) f -> a (four f)", four=CPP)
            .bitcast(fdma)
        )

        # --- scale pipeline ---------------------------------------------
        # partition p holds classes p (cols 0:768) and 75+p (cols 768:1536)
        at2 = attn_pool.tile([KH, 2 * C], f32)
        # FIRST on the Sync ring — see module docstring.
        nc.sync.dma_start(out=at2[:, 0:C], in_=ca[0:KH])
        nc.sync.dma_start(out=at2[:, C : 2 * C], in_=ca[KH : 2 * KH])
        e2 = attn_pool.tile([KH, 2 * C], f32)
        s2 = attn_pool.tile([KH, 2], f32)
        r2 = attn_pool.tile([KH, 2], f32)
        # e2 viewed as (cls-pair, class-half, 192 channel-quads, 4)
        e2_r = e2.rearrange("k (two q m) -> k two q m", two=2, m=CPP)
        # Round 0 (classes 0..74) matmuls overlap the round-1 exp on ACT.
        for rnd in range(2):
            nc.scalar.activation(
                out=e2[:, rnd * C : (rnd + 1) * C],
                in_=at2[:, rnd * C : (rnd + 1) * C],
                func=Exp,
                accum_out=s2[:, rnd : rnd + 1],
            )
            nc.vector.reciprocal(
                out=r2[:, rnd : rnd + 1], in_=s2[:, rnd : rnd + 1]
            )
            for h in range(2):
                for m in range(CPP):
                    col = PSUM_BANK * (CPP * h + m)
                    nc.tensor.matmul(
                        bigpsum[:, col : col + 1],
                        lhsT=e2_r[:, rnd, 96 * h : 96 * (h + 1), m],
                        rhs=r2[:, rnd : rnd + 1],
                        start=(rnd == 0),
                        stop=(rnd == 1),
                    )
        # One strided copy: column 0 of each PSUM bank -> SBUF (96, 8).
        nc.scalar.copy(
            out=scales_all,
            in_=bigpsum.rearrange("p (b c) -> p b c", c=PSUM_BANK)[:, :, 0],
        )

        # --- main scaled copy -------------------------------------------
        # Tile 0 is split 64+32 rows (bases 0/64) so the first store enters
        # the ring right after the scale preamble; tiles 1-3 are full 96.
        def do_rows(xt, p0, pn, r0, col):
            prow = slice(p0, p0 + pn)
            rows = slice(r0, r0 + pn)
            nc.sync.dma_start(out=xt[prow].bitcast(fdma), in_=xf[rows])
            for m in range(CPP):
                nc.vector.tensor_scalar_mul(
                    xt[prow, m * F : (m + 1) * F],
                    xt[prow, m * F : (m + 1) * F],
                    scales_all[prow, col + m : col + m + 1],
                )
            nc.scalar.dma_start(out=of[rows], in_=xt[prow].bitcast(fdma))

        for i in range(N_TILES):
            xt = xpool.tile([P_T, F4], mybir.dt.bfloat16, name="xt", tag="xt")
            col = CPP * (i % 2)
            if i == 0:
                do_rows(xt, 0, 32, 0, col)
                do_rows(xt, 32, 32, 32, col)
                do_rows(xt, 64, 32, 64, col)
            else:
                do_rows(xt, 0, 64, P_T * i, col)
                do_rows(xt, 64, 32, P_T * i + 64, col)


def _get_module():
    if "nc" in _module_cache:
        return _module_cache["nc"]
    nc = bacc.Bacc(
        "TRN2", target_bir_lowering=False, debug=False, enable_asserts=False
    )
    x = nc.dram_tensor(
        "x", (B_SH, C, H, W), mybir.dt.bfloat16, kind="ExternalInput"
    ).ap()
    ca = nc.dram_tensor(
        "channel_attention", (K_CLS, C), mybir.dt.float32, kind="ExternalInput"
    ).ap()
    out = nc.dram_tensor(
        "out", (B_SH, C, H, W), mybir.dt.bfloat16, kind="ExternalOutput"
    ).ap()
    with tile.TileContext(nc) as tc:
        _body(tc, out, x, ca)
    nc.compile()
    _module_cache["nc"] = nc
    return nc


def _run(x, channel_attention, **spmd_kwargs):
    x = np.ascontiguousarray(np.asarray(x, dtype=np.float32))
    ca = np.ascontiguousarray(np.asarray(channel_attention, dtype=np.float32))
    assert x.shape == (B, C, H, W), x.shape
    assert ca.shape == (K_CLS, C), ca.shape
    xb = x.astype(ml_dtypes.bfloat16)
    nc = _get_module()
    in_maps = [
        {"x": xb[i * B_SH : (i + 1) * B_SH], "channel_attention": ca}
        for i in range(N_CORES)
    ]
    res = bass_utils.run_bass_kernel_spmd(
        nc, in_maps, core_ids=list(range(N_CORES)), **spmd_kwargs
    )
    out = np.concatenate([r["out"] for r in res.results], axis=0).astype(np.float32)
    return out, res


def kernel(x, channel_attention):
    out, _ = _run(x, channel_attention)
    return out


# revision 21
# speedup vs baseline: 1.4425x; 1.4425x over previous
"""Trainium2 Bass kernel for nn_ClassChannelAttention.

Computes: out = x * scale[None, :, None, None] where
  scale[c] = sum_k softmax(channel_attention, axis=-1)[k, c]

Sharding: data-parallel over batch B=16 across 8 cores (2 batches/core);
channel_attention (150, 768) replicated to every core. The softmax+class-sum
is tiny and recomputed on each core (no collectives needed).

Precision: the kernel streams x in/out as bf16 (host converts fp32->bf16 on
the way in and upcasts on the way out; the channel scale itself stays fp32
end-to-end on device). This halves HBM traffic per core (50.3 MB -> 25.2 MB)
— the kernel is purely HBM-bandwidth-bound — at a ~2.3e-3 rel-l2 cost, far
under the 2e-2 gate.

DMA schedule (measured across seven variants, 96-121us): per-core HBM runs
~220 GB/s read-only / ~330 GB/s store-only / ~360-400 GB/s mixed, and
end-to-end time grows MONOTONICALLY with store-DMA count (4 stores: 96-99us;
5: 99.7; 7: 101.4; 8: 120.8) — store packets round-robin 1:1 against load
packets per engine and steal bandwidth from the scarcer load direction. So:
4 monolithic (96, 16384) tiles, one load + one store each, 32 KiB rows
(24/48 KiB descriptor geometries measured no better). Loads ride the Sync
HWDGE ring, stores the Scalar ring. The channel_attention load is issued
FIRST on the Sync ring: rings drain FIFO, so it lands in ~3.4us before the
x loads; on the other ring it would round-robin packet-by-packet against
bulk x loads and not land for ~30us, stalling the whole scale pipeline
(measured). Big DMAs are bitcast to int32 (same bytes, 4-byte-typed
descriptors like the fp32 baseline).

Layout: x viewed as (384, 16384) bf16 — FOUR consecutive channel rows per
32 KiB partition row — in 4 tiles of (96, 16384); tile i covers
channel-quads q = 96*(i%2)+p.

Scale pipeline (all preamble, ~12us): exp on ACT (no max-subtraction — ca
is N(0,1), fp32 exp cannot overflow) with fused row-sums, DVE reciprocal,
then the softmax normalization and class-sum fold into tiny PE matmuls:
psum[4h+m][p, 0] = sum_k e[k, 4*(96h+p)+m] * recip[k]  (lhsT = strided e
view, rhs = recip column). Each of the 8 (h, m) outputs gets its OWN psum
tile: PSUM accumulation groups are bank-granular, so concurrent start/stop
groups must live in distinct banks (column-slicing one psum tile corrupts
the sums — caught by CoreSim). Scales are then copied to SBUF fp32 tiles:
the DVE tensor_scalar per-partition scalar must come from SBUF to keep the
4x_2p packed mode (a PSUM-sourced scalar drops the multiply to 1x on HW,
3.8x slower end-to-end — measured). Quarter m of x-tile i is scaled by
scales[i % 2][:, m] (bf16 data, step-1, 4B-aligned -> 4x_2p, ~1.1us/quarter).
"""

import numpy as np
import ml_dtypes

import concourse.bacc as bacc
import concourse.mybir as mybir
import concourse.tile as tile
from concourse import bass_utils

N_CORES = 8
B, C, H, W = 16, 768, 64, 64
K_CLS = 150
B_SH = B // N_CORES          # 2 batches per core
F = H * W                    # 4096
P = 128
CPP = 4                      # channels packed per partition row (32 KiB bf16)
ROWS4 = B_SH * C // CPP      # 384 rows in the merged view
P_T = 128                    # partitions per tile
N_TILES = ROWS4 // P_T       # 3 tiles of (128, 16384) per core
F4 = CPP * F                 # 16384
X_BUFS = 3                   # all 3 x tiles in flight

_module_cache = {}


def _body(tc, out, x, ca):
    nc = tc.nc
    f32 = mybir.dt.float32
    Exp = mybir.ActivationFunctionType.Exp

    with (
        tc.tile_pool(name="attn", bufs=2) as attn_pool,
        tc.tile_pool(name="small", bufs=1) as small,
        tc.tile_pool(name="psum", bufs=1, space="PSUM") as psum_pool,
        tc.tile_pool(name="xt", bufs=X_BUFS) as xpool,
    ):
        # Tile i covers channel-quads q = (128i + p) mod 192:
        # S0[p,m]=scale[4p+m] (tile 0), S2[p,m]=scale[4(64+p)+m] (tile 2),
        # S1 = straddle for tile 1: [0:64]=scale[4(128+p)+m] (=S2[64+p]),
        # [64:128]=scale[4(p-64)+m] (=S0[p-64]) — recomputed by a second
        # sequential matmul batch into the correct partitions (engines
        # cannot move values across partitions).
        scales = [
            small.tile([P_T, CPP], f32, name=f"scale{h}", tag=f"scale{h}")
            for h in range(3)
        ]
        psums = [
            psum_pool.tile([P_T, 1], f32, name=f"ps{k}", tag=f"ps{k}")
            for k in range(2 * CPP)
        ]

        i32 = mybir.dt.int32
        xf = (
            x.rearrange("b c h w -> (b c) (h w)")
            .rearrange("(a four) f -> a (four f)", four=CPP)
            .bitcast(i32)
        )
        of = (
            out.rearrange("b c h w -> (b c) (h w)")
            .rearrange("(a four) f -> a (four f)", four=CPP)
            .bitcast(i32)
        )

        # Softmax over channels per class; classes on partitions (128 + 22).
        row_splits = [(0, 128), (128, K_CLS - 128)]
        ers, rs = [], []
        for idx, (r0, rn) in enumerate(row_splits):
            at = attn_pool.tile([P, C], f32, tag="attn")
            # FIRST on the Sync ring — see module docstring.
            nc.sync.dma_start(out=at[:rn], in_=ca[r0 : r0 + rn])
            e = attn_pool.tile([P, C], f32, tag="e")
            s = attn_pool.tile([P, 1], f32, tag="s")
            # e = exp(at); s = per-class row sum of e (fused accum).
            nc.scalar.activation(out=e[:rn], in_=at[:rn], func=Exp, accum_out=s[:rn])
            r = attn_pool.tile([P, 1], f32, tag="r")
            nc.vector.reciprocal(out=r[:rn], in_=s[:rn])
            # Class-sum of softmax into channel-on-partition layouts via tiny
            # matmuls; rhs = recip folds the softmax normalization in.
            # e viewed as (cls, 192 channel-quads, 4).
            e_r = e.rearrange("k (q m) -> k q m", m=CPP)
            ers.append((e_r, rn))
            rs.append(r)
            # Batch 1: S0 (banks 0-3) and S2 (banks 4-7).
            for m in range(CPP):
                for bank, q0 in ((m, 0), (CPP + m, 64)):
                    nc.tensor.matmul(
                        psums[bank],
                        lhsT=e_r[:rn, q0 : q0 + P_T, m],
                        rhs=r[:rn],
                        start=(idx == 0),
                        stop=(idx == len(row_splits) - 1),
                    )
        for m in range(CPP):
            nc.scalar.copy(out=scales[0][:, m : m + 1], in_=psums[m])
            nc.scalar.copy(out=scales[2][:, m : m + 1], in_=psums[CPP + m])
        # Batch 2 (sequential bank reuse after the copies): tile 1 straddle.
        for idx in range(2):
            e_r, rn = ers[idx]
            r = rs[idx]
            for m in range(CPP):
                nc.tensor.matmul(
                    psums[m][0:64],
                    lhsT=e_r[:rn, 128:192, m],
                    rhs=r[:rn],
                    start=(idx == 0),
                    stop=(idx == 1),
                )
                nc.tensor.matmul(
                    psums[CPP + m][64:128],
                    lhsT=e_r[:rn, 0:64, m],
                    rhs=r[:rn],
                    start=(idx == 0),
                    stop=(idx == 1),
                )
        for m in range(CPP):
            nc.scalar.copy(out=scales[1][0:64, m : m + 1], in_=psums[m][0:64])
            nc.scalar.copy(
                out=scales[1][64:128, m : m + 1], in_=psums[CPP + m][64:128]
            )

        # Main scaled copy: 4 tiles of (96, 16384) bf16; quarter m of tile i
        # scaled by scales[i % 2][:, m] (DVE 4x_2p, ~1.1us/quarter — ~17us
        # total DVE, fully hidden under the ~80us DMA window).
        for i in range(N_TILES):
            sel = scales[i]
            rows = slice(i * P_T, (i + 1) * P_T)
            xt = xpool.tile([P_T, F4], mybir.dt.bfloat16, name="xt", tag="xt")
            nc.sync.dma_start(out=xt.bitcast(i32), in_=xf[rows])
            # Tile 1's map straddles q=192: multiply per 64-partition half
            # (bases 0/64) so each half reads its lane-aligned scale column.
            pranges = [slice(0, 64), slice(64, 128)] if i == 1 else [slice(0, P_T)]
            for pr in pranges:
                for m in range(CPP):
                    nc.vector.tensor_scalar_mul(
                        xt[pr, m * F : (m + 1) * F],
                        xt[pr, m * F : (m + 1) * F],
                        sel[pr, m : m + 1],
                    )
            nc.scalar.dma_start(out=of[rows], in_=xt.bitcast(i32))


def _get_module():
    if "nc" in _module_cache:
        return _module_cache["nc"]
    nc = bacc.Bacc(
        "TRN2", target_bir_lowering=False, debug=False, enable_asserts=False
    )
    x = nc.dram_tensor(
        "x", (B_SH, C, H, W), mybir.dt.bfloat16, kind="ExternalInput"
    ).ap()
    ca = nc.dram_tensor(
        "channel_attention", (K_CLS, C), mybir.dt.float32, kind="ExternalInput"
    ).ap()
    out = nc.dram_tensor(
        "out", (B_SH, C, H, W), mybir.dt.bfloat16, kind="ExternalOutput"
    ).ap()
    with tile.TileContext(nc) as tc:
        _body(tc, out, x, ca)
    nc.compile()
    _module_cache["nc"] = nc
    return nc


def _run(x, channel_attention, **spmd_kwargs):
    x = np.ascontiguousarray(np.asarray(x, dtype=np.float32))
    ca = np.ascontiguousarray(np.asarray(channel_attention, dtype=np.float32))
    assert x.shape == (B, C, H, W), x.shape
    assert ca.shape == (K_CLS, C), ca.shape
    xb = x.astype(ml_dtypes.bfloat16)
    nc = _get_module()
    in_maps = [
        {"x": xb[i * B_SH : (i + 1) * B_SH], "channel_attention": ca}
        for i in range(N_CORES)
    ]
    res = bass_utils.run_bass_kernel_spmd(
        nc, in_maps, core_ids=list(range(N_CORES)), **spmd_kwargs
    )
    out = np.concatenate([r["out"] for r in res.results], axis=0).astype(np.float32)
    return out, res


def kernel(x, channel_attention):
    out, _ = _run(x, channel_attention)
    return out
